# revision 7
# baseline (speedup 1.0000x reference)
"""Trainium2 Bass kernel for nn_ContextAttention_30270929502708.

Reference computation (N=2, C=64, M=4800, K=64):
  cat   = [scene_rgb (bc over m); query (bc over k)]         (N,2C,M,K)
  h     = relu(bn0(W0 @ cat))
  cat2  = [h; scene_xyz*mask (bc over m)]                    (N,C+3,M,K)
  h1    = relu(bn1a(W1a @ cat2)); h1 = relu(bn1b(W1b @ h1))
  feat  = h1 + Wskip @ cat2
  out   = Wout @ [max_k feat; pre_xyz]                       (N,3,M)

Restructure: every conv contribution from k-only tensors (scene_rgb,
scene_xyz*mask) is a tiny (C,K) constant, precomputed on host with BN
scales folded into the weights:
  h  = relu(A0h[c,k] + B0[c,m]),  B0 = W0q' @ query   (on device)
  h1 = relu(W1a' @ h + D1h[c,k]); h2 = relu(W1b' @ h1 + t1b[c])
  feat_mk = h2 + Wskip_c @ h + Dkh[c,k]
Partition layout packs both batches: p = n*64 + c (block-diag weights).
Per-core m-shard of 600; loop over k with per-k [P,1] scalars; the
max over k is a running scalar_tensor_tensor accumulate on DVE.

h2 is injected into the skip PSUM bank via an identity matmul on PE
(PSUM accumulation REQUIRES a start=True matmul to open the group, so
ACT cannot pre-write the bank). Inject + feat-max are deferred one k
(software pipelining). All prologue constants are packed into two
DMAs (fp32 + bf16 images); query/b0 are bf16 so the DVE h op runs in
2x mode.

Variants (for slope-isolating engine costs on HW):
  full    — the real kernel
  pe_only — only the 8 matmuls/k (constant rhs)
  noact   — PE + DVE ops (h1 bypassed)
  nodve   — PE + ACT ops (h/featmax bypassed)
"""

import functools

import numpy as np

N, C, M, K = 2, 64, 4800, 64
EPS = 1e-5
NCORES = 8
MCORE = M // NCORES  # 600
MH = MCORE // 2      # 300 (matmul half, fits one PSUM bank)

# fp32 constant image columns
A0_OFF, D1_OFF, DK_OFF, T1B_OFF, WOUT_OFF = 0, K, 2 * K, 3 * K, 3 * K + 1
C32_W = 3 * K + 1 + 6  # 199
# bf16 constant image columns
WQ0_OFF, W1A_OFF, W1B_OFF, WSK_OFF, WOX_OFF = 0, 128, 256, 384, 512
IDT_OFF = 518
C16_W = 4 * 128 + 6 + 128    # 646


def _blkdiag(a, b):
    out = np.zeros((a.shape[0] + b.shape[0], a.shape[1] + b.shape[1]), np.float32)
    out[: a.shape[0], : a.shape[1]] = a
    out[a.shape[0] :, a.shape[1] :] = b
    return out


def _fold(g, b, m, v):
    s = g / np.sqrt(v + EPS)
    return s.astype(np.float32), (b - m * s).astype(np.float32)


@functools.lru_cache(maxsize=16)
def _build_program(bench_reps=1, variant="full"):
    import concourse.mybir as mybir
    import concourse.tile as tile
    from concourse import bacc

    fp32 = mybir.dt.float32
    bf16 = mybir.dt.bfloat16
    AT = mybir.ActivationFunctionType
    OP = mybir.AluOpType

    nc = bacc.Bacc("TRN2", target_bir_lowering=False, debug=False,
                   num_devices=NCORES)

    din = {
        "c32": nc.dram_tensor("c32", [128, C32_W], fp32, kind="ExternalInput"),
        "cb16": nc.dram_tensor("cb16", [128, C16_W], bf16, kind="ExternalInput"),
        "query_s": nc.dram_tensor("query_s", [128, MCORE], bf16,
                                  kind="ExternalInput"),
        "prexyz_s": nc.dram_tensor("prexyz_s", [6, MCORE], bf16,
                                   kind="ExternalInput"),
    }
    out_s = nc.dram_tensor("out_s", [6, MCORE], fp32, kind="ExternalOutput")

    with tile.TileContext(nc) as tc:
        with (
            tc.tile_pool(name="const", bufs=1) as cp,
            tc.tile_pool(name="hp", bufs=3) as hp,
            tc.tile_pool(name="h1p", bufs=2) as h1p,
            tc.tile_pool(name="h2p", bufs=3) as h2p,
            tc.tile_pool(name="pp1", bufs=1, space="PSUM") as pp1,
            tc.tile_pool(name="pp2", bufs=1, space="PSUM") as pp2,
            tc.tile_pool(name="pp3", bufs=2, space="PSUM") as pp3,
        ):
            # ---- load constants / per-core inputs into SBUF (4 DMAs) ----
            c32 = cp.tile([128, C32_W], fp32, tag="c32")
            nc.sync.dma_start(out=c32, in_=din["c32"][:, :])
            cb16 = cp.tile([128, C16_W], bf16, tag="cb16")
            nc.sync.dma_start(out=cb16, in_=din["cb16"][:, :])
            query = cp.tile([128, 2, MH], bf16, tag="query_s")
            nc.sync.dma_start(
                out=query,
                in_=din["query_s"][:, :].rearrange("p (a b) -> p a b", a=2))
            prexyz = cp.tile([6, 2, MH], bf16, tag="prexyz_s")
            nc.sync.dma_start(
                out=prexyz,
                in_=din["prexyz_s"][:, :].rearrange("p (a b) -> p a b", a=2))

            def w16(off):
                return cb16[:, off : off + 128]

            # ---- B0 = Wq0' @ query  (PSUM -> SBUF, bf16) ----
            pb = pp1.tile([128, 2, 512], fp32, tag="p1")
            for i in range(2):
                nc.tensor.matmul(out=pb[:, i, :MH], lhsT=w16(WQ0_OFF),
                                 rhs=query[:, i, :], start=True, stop=True)
            b0 = cp.tile([128, 2, MH], bf16, tag="b0")
            nc.scalar.copy(out=b0, in_=pb[:, :, :MH])

            # ---- feat ping-pong accumulators ----
            feats = [cp.tile([128, 2, MH], fp32, tag=f"feat{i}", name=f"feat{i}")
                     for i in range(2)]
            nc.vector.memset(feats[0], -1e30)

            # ---- main loop over k ----
            # bench_reps>1 unrolls extra sweeps: the feat max-accumulate is
            # idempotent, so repeats are numerically safe and expose the
            # loop's HW time as a slope vs reps.
            # inject[k] + feat-max[k] are deferred by one k (software
            # pipelining) so PE never stalls waiting for ACT's h2[k].
            use_act = variant in ("full", "nodve")
            use_dve = variant in ("full", "noact")
            pending = []

            def flush_pending():
                p3_p, h2_p, k_p = pending.pop()
                for i in range(2):
                    nc.tensor.matmul(out=p3_p[:, i, :MH], lhsT=w16(IDT_OFF),
                                     rhs=h2_p[:, i, :], start=False, stop=True)
                if use_dve:
                    s, d = feats[k_p % 2], feats[(k_p + 1) % 2]
                    nc.vector.scalar_tensor_tensor(
                        out=d, in0=p3_p[:, :, :MH],
                        scalar=c32[:, DK_OFF + k_p : DK_OFF + k_p + 1],
                        in1=s, op0=OP.add, op1=OP.max)

            for _rep in range(bench_reps):
                for k in range(K):
                    if use_dve:
                        # h = relu(B0 + a0h[:,k]) (DVE, all-bf16: 2x mode)
                        h = hp.tile([128, 2, MH], bf16, tag="h", name="h")
                        nc.vector.tensor_scalar(
                            out=h, in0=b0,
                            scalar1=c32[:, A0_OFF + k : A0_OFF + k + 1],
                            scalar2=0.0, op0=OP.add, op1=OP.max)
                    else:
                        h = b0

                    p1 = pp1.tile([128, 2, 512], fp32, tag="p1", name="p1")
                    for i in range(2):
                        nc.tensor.matmul(out=p1[:, i, :MH], lhsT=w16(W1A_OFF),
                                         rhs=h[:, i, :], start=True, stop=True)
                    if use_act:
                        h1 = h1p.tile([128, 2, MH], bf16, tag="h1", name="h1")
                        nc.scalar.activation(
                            out=h1, in_=p1[:, :, :MH], func=AT.Relu,
                            bias=c32[:, D1_OFF + k : D1_OFF + k + 1], scale=1.0)
                    else:
                        h1 = h

                    p2 = pp2.tile([128, 2, 512], fp32, tag="p2", name="p2")
                    for i in range(2):
                        nc.tensor.matmul(out=p2[:, i, :MH], lhsT=w16(W1B_OFF),
                                         rhs=h1[:, i, :], start=True, stop=True)
                    if use_act:
                        h2 = h2p.tile([128, 2, MH], bf16, tag="h2", name="h2")
                        nc.scalar.activation(
                            out=h2, in_=p2[:, :, :MH], func=AT.Relu,
                            bias=c32[:, T1B_OFF : T1B_OFF + 1], scale=1.0)
                    else:
                        h2 = h

                    p3 = pp3.tile([128, 2, 512], fp32, tag="p3", name="p3")
                    for i in range(2):
                        nc.tensor.matmul(out=p3[:, i, :MH], lhsT=w16(WSK_OFF),
                                         rhs=h[:, i, :], start=True, stop=False)
                    if pending:
                        flush_pending()
                    pending.append((p3, h2, k))
                if pending:
                    flush_pending()

            feat = feats[K % 2]
            # ---- out conv: Wout_c @ feat + Wout_x @ pre_xyz ----
            po = pp2.tile([6, 2, 512], fp32, tag="p2", name="po")
            for i in range(2):
                nc.tensor.matmul(out=po[:, i, :MH],
                                 lhsT=c32[:, WOUT_OFF : WOUT_OFF + 6],
                                 rhs=feat[:, i, :], start=True, stop=False)
                nc.tensor.matmul(out=po[:, i, :MH],
                                 lhsT=cb16[0:6, WOX_OFF : WOX_OFF + 6],
                                 rhs=prexyz[:, i, :], start=False, stop=True)
            out_sb = cp.tile([6, 2, MH], fp32, tag="out_sb")
            nc.scalar.copy(out=out_sb, in_=po[:, :, :MH])
            nc.sync.dma_start(out=out_s[:, :].rearrange("p (a b) -> p a b", a=2),
                              in_=out_sb)

    nc.compile()
    return nc


def _host_prep(query_rgb_feat, scene_rgb_feat, scene_xyz, pre_xyz, mask,
               W0, g0, b0, m0, v0, W1a, g1a, b1a, m1a, v1a,
               W1b, g1b, b1b, m1b, v1b, Wskip, Wout):
    f32 = np.float32
    s0, t0 = _fold(g0, b0, m0, v0)
    s1a, t1a = _fold(g1a, b1a, m1a, v1a)
    s1b, t1b = _fold(g1b, b1b, m1b, v1b)

    scene = np.asarray(scene_rgb_feat, f32)[:, :, 0, :]          # (N,C,K)
    sxm = (np.asarray(scene_xyz, f32) * np.asarray(mask, f32))[:, :, 0, :]  # (N,3,K)
    query = np.asarray(query_rgb_feat, f32)[:, :, :, 0]          # (N,C,M)

    W0 = np.asarray(W0, f32)
    W1a = np.asarray(W1a, f32)
    W1b = np.asarray(W1b, f32)
    Wskip = np.asarray(Wskip, f32)
    Wout = np.asarray(Wout, f32)

    # per-batch (C,K) constants with BN folded
    a0h = np.concatenate(
        [s0[:, None] * (W0[:, :C] @ scene[n]) + t0[:, None] for n in range(N)], 0)
    d1h = np.concatenate(
        [s1a[:, None] * (W1a[:, C:] @ sxm[n]) + t1a[:, None] for n in range(N)], 0)
    dkh = np.concatenate([Wskip[:, C:] @ sxm[n] for n in range(N)], 0)

    w0q = s0[:, None] * W0[:, C:]
    w1a_c = s1a[:, None] * W1a[:, :C]
    w1b_c = s1b[:, None] * W1b

    import ml_dtypes
    bf16 = ml_dtypes.bfloat16

    c32 = np.zeros((128, C32_W), f32)
    c32[:, A0_OFF : A0_OFF + K] = a0h
    c32[:, D1_OFF : D1_OFF + K] = d1h
    c32[:, DK_OFF : DK_OFF + K] = dkh
    c32[:, T1B_OFF] = np.tile(t1b, 2)
    c32[:, WOUT_OFF : WOUT_OFF + 6] = _blkdiag(Wout[:, :C].T, Wout[:, :C].T)

    cb16 = np.zeros((128, C16_W), f32)
    cb16[:, WQ0_OFF : WQ0_OFF + 128] = _blkdiag(w0q.T, w0q.T)
    cb16[:, W1A_OFF : W1A_OFF + 128] = _blkdiag(w1a_c.T, w1a_c.T)
    cb16[:, W1B_OFF : W1B_OFF + 128] = _blkdiag(w1b_c.T, w1b_c.T)
    cb16[:, WSK_OFF : WSK_OFF + 128] = _blkdiag(Wskip[:, :C].T, Wskip[:, :C].T)
    cb16[0:6, WOX_OFF : WOX_OFF + 6] = _blkdiag(Wout[:, C:].T, Wout[:, C:].T)
    cb16[:, IDT_OFF : IDT_OFF + 128] = np.eye(128, dtype=f32)

    consts = {
        "c32": np.ascontiguousarray(c32, f32),
        "cb16": np.ascontiguousarray(cb16).astype(bf16),
    }
    query_p = query.reshape(N * C, M).astype(bf16)            # (128, M)
    prexyz_p = np.asarray(pre_xyz, f32).reshape(N * 3, M).astype(bf16)  # (6, M)
    return consts, query_p, prexyz_p


def _run_via_pjrt(nc, in_maps, bench_iters=0, _return_fn=False):
    """Execute the Bass module on NCORES cores via PJRT (axon-friendly).

    Mirrors bass2jax.run_bass_via_pjrt's multi-core path but keeps the
    jitted callable so repeated timed executions are possible.
    Returns (per_core_results, per_iter_seconds_list).
    """
    import time

    import jax
    import jax.numpy as jnp
    from jax.sharding import Mesh, NamedSharding, PartitionSpec
    from jax.experimental.shard_map import shard_map

    import concourse.mybir as mybir
    from concourse import bass2jax

    bass2jax.install_neuronx_cc_hook()
    assert nc.dbg_addr is None
    partition_name = (nc.partition_id_tensor.name
                      if nc.partition_id_tensor else None)

    in_names, out_names, out_avals, zero_outs = [], [], [], []
    for alloc in nc.m.functions[0].allocations:
        if not isinstance(alloc, mybir.MemoryLocationSet):
            continue
        name = alloc.memorylocations[0].name
        if alloc.kind == "ExternalInput":
            if name != partition_name:
                in_names.append(name)
        elif alloc.kind == "ExternalOutput":
            shape = tuple(alloc.tensor_shape)
            dtype = mybir.dt.np(alloc.dtype)
            out_names.append(name)
            out_avals.append(jax.core.ShapedArray(shape, dtype))
            zero_outs.append(np.zeros(shape, dtype))
    n_params = len(in_names)
    n_outs = len(out_avals)
    all_in_names = in_names + out_names
    if partition_name is not None:
        all_in_names.append(partition_name)
    donate = tuple(range(n_params, n_params + n_outs))

    def _body(*args):
        operands = list(args)
        if partition_name is not None:
            operands.append(bass2jax.partition_id_tensor())
        outs = bass2jax._bass_exec_p.bind(
            *operands,
            out_avals=tuple(out_avals),
            in_names=tuple(all_in_names),
            out_names=tuple(out_names),
            lowering_input_output_aliases=(),
            sim_require_finite=True,
            sim_require_nnan=True,
            nc=nc,
        )
        return tuple(outs)

    devices = jax.devices()[:NCORES]
    mesh = Mesh(np.asarray(devices), ("core",))
    spec = PartitionSpec("core")
    in_specs = (spec,) * (n_params + n_outs)
    out_specs = (spec,) * n_outs
    sharded = jax.jit(
        shard_map(_body, mesh=mesh, in_specs=in_specs, out_specs=out_specs,
                  check_rep=False),
        donate_argnums=donate, keep_unused=True)

    concat_in = [
        np.concatenate([np.asarray(in_maps[c][name]) for c in range(NCORES)], 0)
        for name in in_names
    ]
    concat_zero_shapes = [(NCORES * z.shape[0], *z.shape[1:]) for z in zero_outs]
    sh = NamedSharding(mesh, spec)
    dev_in = [jax.device_put(a, sh) for a in concat_in]

    def _zeros():
        zs = [jax.device_put(np.zeros(s, np.float32), sh)
              for s in concat_zero_shapes]
        jax.block_until_ready(zs)
        return zs

    out_arrs = sharded(*dev_in, *_zeros())
    jax.block_until_ready(out_arrs)

    def timed_once():
        zs = _zeros()
        t0 = time.perf_counter()
        r = sharded(*dev_in, *zs)
        jax.block_until_ready(r)
        return time.perf_counter() - t0

    times = [timed_once() for _ in range(bench_iters)]

    results = [
        {name: np.asarray(out_arrs[i]).reshape(NCORES, *out_avals[i].shape)[c]
         for i, name in enumerate(out_names)}
        for c in range(NCORES)
    ]
    if _return_fn:
        return results, times, timed_once
    return results, times


def _make_in_maps(inputs):
    consts, query_p, prexyz_p = _host_prep(**inputs)
    in_maps = []
    for c in range(NCORES):
        sl = slice(c * MCORE, (c + 1) * MCORE)
        im = dict(consts)
        im["query_s"] = np.ascontiguousarray(query_p[:, sl])
        im["prexyz_s"] = np.ascontiguousarray(prexyz_p[:, sl])
        in_maps.append(im)
    return in_maps


def _assemble(results):
    out = np.empty((N, 3, M), np.float32)
    for c in range(NCORES):
        sl = slice(c * MCORE, (c + 1) * MCORE)
        out[:, :, sl] = results[c]["out_s"].reshape(N, 3, MCORE)
    return out


def kernel(**inputs):
    nc = _build_program(1)
    in_maps = _make_in_maps(inputs)
    results, _ = _run_via_pjrt(nc, in_maps, bench_iters=0)
    return _assemble(results)


def _make_runner(nc, in_maps):
    """Build the jitted sharded callable once; return (run_once, results_fn)."""
    import time

    import jax

    state = {}

    def run_once():
        if "fn" not in state:
            results, times, fn = _run_via_pjrt(nc, in_maps, bench_iters=1,
                                               _return_fn=True)
            state["fn"] = fn
            state["results"] = results
            return times[0]
        return state["fn"]()

    return run_once, lambda: state["results"]


def bench_loop_ns(inputs, r_lo=1, r_hi=33, iters=30, verbose=False,
                  variant="full"):
    """Estimate the HW time of one 64-k main-loop sweep via the slope of
    wall time vs in-kernel unrolled repetition count. Paired back-to-back
    (lo, hi) runs with a median over the differences cancel axon RPC
    drift and outliers."""
    in_maps = _make_in_maps(inputs)
    run_lo, _ = _make_runner(_build_program(r_lo, variant), in_maps)
    run_hi, _ = _make_runner(_build_program(r_hi, variant), in_maps)
    diffs = []
    for i in range(iters):
        if i % 2 == 0:
            a, b = run_lo(), run_hi()
            diffs.append(b - a)
        else:
            b, a = run_hi(), run_lo()
            diffs.append(b - a)
    diffs_ns = np.array(diffs) / (r_hi - r_lo) * 1e9
    if verbose:
        print("per-pair slope estimates (ns):",
              np.percentile(diffs_ns, [10, 25, 50, 75, 90]).astype(int))
    return float(np.median(diffs_ns))
